# revision 1
# baseline (speedup 1.0000x reference)
"""MoE layer (top-2 routing, 8 experts) on 8 Trainium2 NeuronCores.

Strategy (expert-parallel, per sharding hint):
  - Host computes the gate (replicated router math in fp64 numpy): logits,
    top-2 experts per token, softmax gates.
  - Tokens are dispatched (host-side all-to-all) into per-expert batches,
    padded to a common capacity; core c holds expert c's weights and its
    token batch.
  - Each core runs the expert MLP: Y^T = g ⊙ (relu(W1^T X^T + b1) W2)^T in
    a feature-major (transposed) dataflow, bf16 matmuls with fp32 PSUM
    accumulation.
  - Host combines: out[tok] += Y rows (+ g * b2), summing each token's two
    expert contributions.

Hardcoded problem shape: x(8192,1024) w1(8,1024,4096) w2(8,4096,1024).
"""

import numpy as np
import ml_dtypes

import concourse.bass as bass  # noqa: F401  (bass types referenced via tile/bacc)
import concourse.tile as tile
import concourse.mybir as mybir
from concourse import bacc
from concourse.bass_utils import run_bass_kernel_spmd

E = 8          # experts == cores
D = 1024       # model dim
H = 4096       # hidden dim
TOP_K = 2
N_CORES = 8
ND = D // 128  # 8 d-tiles
NH = H // 128  # 32 h-tiles

F32 = mybir.dt.float32
BF16 = mybir.dt.bfloat16


def _token_tiles(cap):
    tiles = []
    t = 0
    while t < cap:
        n = min(512, cap - t)
        tiles.append((t, n))
        t += n
    return tiles


def build_moe(cap):
    """Build + compile the per-core expert-MLP Bass program for capacity cap."""
    nc = bacc.Bacc("TRN2", target_bir_lowering=False, debug=False, num_devices=N_CORES)

    xt = nc.dram_tensor("xt", [D, cap], BF16, kind="ExternalInput")      # x[idx].T
    w1 = nc.dram_tensor("w1", [D, H], BF16, kind="ExternalInput")
    w2 = nc.dram_tensor("w2", [H, D], BF16, kind="ExternalInput")
    b1 = nc.dram_tensor("b1", [128, NH], F32, kind="ExternalInput")      # b1[p,j]=b1_full[j*128+p]
    g = nc.dram_tensor("g", [128, cap], F32, kind="ExternalInput")       # gate, replicated rows
    yt = nc.dram_tensor("yt", [D, cap], F32, kind="ExternalOutput")

    xt_ap, w1_ap, w2_ap, b1_ap, g_ap, yt_ap = (
        t.ap() for t in (xt, w1, w2, b1, g, yt)
    )

    with tile.TileContext(nc) as tc:
        with (
            tc.tile_pool(name="wpool", bufs=1) as wpool,
            tc.tile_pool(name="xpool", bufs=16) as xpool,
            tc.tile_pool(name="hpool", bufs=36) as hpool,
            tc.tile_pool(name="ypool", bufs=4) as ypool,
            tc.tile_pool(name="gpool", bufs=2) as gpool,
            tc.tile_pool(name="ph", bufs=4, space="PSUM") as ph_pool,
            tc.tile_pool(name="py", bufs=2, space="PSUM") as py_pool,
        ):
            # Resident weights
            w1_sb = []
            for d in range(ND):
                t = wpool.tile([128, H], BF16, name=f"w1sb{d}", tag=f"w1sb{d}")
                nc.sync.dma_start(t[:], w1_ap[d * 128:(d + 1) * 128, :])
                w1_sb.append(t)
            w2_sb = []
            for h in range(NH):
                t = wpool.tile([128, D], BF16, name=f"w2sb{h}", tag=f"w2sb{h}")
                nc.sync.dma_start(t[:], w2_ap[h * 128:(h + 1) * 128, :])
                w2_sb.append(t)
            b1_sb = wpool.tile([128, NH], F32, name="b1sb", tag="b1sb")
            nc.sync.dma_start(b1_sb[:], b1_ap[:, :])

            for (t0, tn) in _token_tiles(cap):
                g_sb = gpool.tile([128, 512], F32, name=f"gsb{t0}", tag="gsb")
                nc.sync.dma_start(g_sb[:, :tn], g_ap[:, t0:t0 + tn])

                x_sb = []
                for d in range(ND):
                    t = xpool.tile([128, 512], BF16, name=f"xsb{t0}_{d}", tag="xsb")
                    nc.sync.dma_start(t[:, :tn], xt_ap[d * 128:(d + 1) * 128, t0:t0 + tn])
                    x_sb.append(t)

                # Layer 1: H^T[h_tile] = relu(sum_d W1[d,h]^T X^T[d] + b1)
                h_sb = []
                for h in range(NH):
                    ph = ph_pool.tile([128, 512], F32, name=f"ph{t0}_{h}", tag="ph")
                    for d in range(ND):
                        nc.tensor.matmul(
                            ph[:, :tn],
                            w1_sb[d][:, h * 128:(h + 1) * 128],
                            x_sb[d][:, :tn],
                            start=(d == 0),
                            stop=(d == ND - 1),
                        )
                    ht = hpool.tile([128, 512], BF16, name=f"hsb{t0}_{h}", tag="hsb")
                    nc.scalar.activation(
                        ht[:, :tn], ph[:, :tn],
                        mybir.ActivationFunctionType.Relu,
                        bias=b1_sb[:, h:h + 1],
                    )
                    h_sb.append(ht)

                # Layer 2: Y^T[do] = g ⊙ sum_h W2[h,do]^T H^T[h]
                for do in range(ND):
                    py = py_pool.tile([128, 512], F32, name=f"py{t0}_{do}", tag="py")
                    for h in range(NH):
                        nc.tensor.matmul(
                            py[:, :tn],
                            w2_sb[h][:, do * 128:(do + 1) * 128],
                            h_sb[h][:, :tn],
                            start=(h == 0),
                            stop=(h == NH - 1),
                        )
                    y_sb = ypool.tile([128, 512], F32, name=f"ysb{t0}_{do}", tag="ysb")
                    nc.vector.tensor_mul(y_sb[:, :tn], py[:, :tn], g_sb[:, :tn])
                    nc.sync.dma_start(yt_ap[do * 128:(do + 1) * 128, t0:t0 + tn], y_sb[:, :tn])

    nc.compile()
    return nc


def _route(x, wg, bg):
    """Host router in fp64: per-token top-2 experts and softmax gates."""
    logits = x.astype(np.float64) @ wg.astype(np.float64).T + bg.astype(np.float64)
    top2 = np.argpartition(-logits, 1, axis=1)[:, :TOP_K]  # two largest, unordered
    vals = np.take_along_axis(logits, top2, axis=1)
    ex = np.exp(vals - vals.max(axis=1, keepdims=True))
    gates = ex / ex.sum(axis=1, keepdims=True)
    idxs, gs = [], []
    for e in range(E):
        mask = top2 == e
        rows = np.nonzero(mask.any(axis=1))[0]
        idxs.append(rows)
        gs.append(gates[mask].astype(np.float32))
    return idxs, gs


def moe_run(x, wg, bg, w1, b1, w2, b2, trace=False, trace_kwargs=None):
    x = np.ascontiguousarray(np.asarray(x, np.float32))
    wg = np.asarray(wg, np.float32)
    bg = np.asarray(bg, np.float32)
    w1 = np.asarray(w1, np.float32)
    b1 = np.asarray(b1, np.float32)
    w2 = np.asarray(w2, np.float32)
    b2 = np.asarray(b2, np.float32)
    B = x.shape[0]

    idxs, gs = _route(x, wg, bg)
    cap = max(256, -(-max(len(r) for r in idxs) // 128) * 128)

    nc = build_moe(cap)

    in_maps = []
    for e in range(E):
        n = len(idxs[e])
        xe = np.zeros((cap, D), np.float32)
        xe[:n] = x[idxs[e]]
        ge = np.zeros((cap,), np.float32)
        ge[:n] = gs[e]
        in_maps.append({
            "xt": np.ascontiguousarray(xe.T).astype(ml_dtypes.bfloat16),
            "w1": w1[e].astype(ml_dtypes.bfloat16),
            "w2": w2[e].astype(ml_dtypes.bfloat16),
            "b1": np.ascontiguousarray(b1[e].reshape(NH, 128).T),
            "g": np.ascontiguousarray(np.broadcast_to(ge, (128, cap))),
        })

    kwargs = {}
    if trace:
        kwargs["trace"] = True
        if trace_kwargs:
            kwargs.update(trace_kwargs)
    res = run_bass_kernel_spmd(nc, in_maps, core_ids=list(range(N_CORES)), **kwargs)

    out = np.zeros((B, D), np.float32)
    for e in range(E):
        n = len(idxs[e])
        y = res.results[e]["yt"][:, :n].T  # (n, D), gate already applied
        out[idxs[e]] += y + gs[e][:, None] * b2[e][None, :]
    return out, res


def kernel(x, wg, bg, w1, b1, w2, b2):
    out, _ = moe_run(x, wg, bg, w1, b1, w2, b2, trace=False)
    return out
